# revision 9
# baseline (speedup 1.0000x reference)
"""LIF neuron scan kernel v4 for Trainium2 (8 NeuronCores).

Reference recurrence per timestep t (elementwise over B x N):
    u = (x_t - m)*sig + m ; s = (u >= th) ; m = (1-s)*u
with sig = sigmoid(tau_x) in (0,1), th > 0 per neuron.

uh-space (v3) reformulation kept: uh = u/th, xs = x*(sig/th), cm = 1-sig:
    w  = (uh < 1) * cm        # stt, fp32 (w==0 iff spike; w carries the decay)
    p  = uh * w               # tt fp32
    uh'= p + xs'              # tt fp32
State stays fp32 end-to-end: fp16-state variants flip ~2-3k near-threshold
spikes (emulated: 2.2-2.6e-2 rel err > the 2e-2 gate), so 16-bit state is
not usable; only non-state paths are 16-bit.

v4 changes vs v3 (probe-informed; see probe.py):
  - Input pipeline: one 2 MB dma_start per 8-t block, 3-slot ring with a
    per-slot semaphore (no cross-block issue throttle). v3 serialized input
    DMAs behind output waits + a completion throttle and DVE idled 4.8 us
    per block waiting for x.
  - mems path: ACT casts the p block to fp16 (ACT is ~70% idle), DVE does
    mems16 = p16 * thcm16 as an all-fp16 tensor_tensor -> 2x_1P mode
    (2.2 us vs 4.3 us per block). mems output fp16 (was bf16): same bytes,
    ~8x better mantissa. Accuracy impact ~5e-4 rel, well inside the gate.
  - spikes: one ACT Exp per block ([128,4096], fp8e4 out, exact 0/1),
    halving spike DMA bytes and cutting per-t semaphore traffic.
  - Chain ops carry no per-t then_inc except the block-final ones: pure TT
    pitch is 602 ns; v3's per-op sem traffic ran it at 722 ns.
  - GpSimd compute offload was probed and rejected: a gpsimd tensor_tensor
    blocks concurrent DVE almost completely (one DVE op per gpsimd op).
    SWDGE DMA traffic does NOT block DVE (probed) but is not needed.
  - DMA accum_op=mult (CCE) is rejected by the compiler ("DMACopy does not
    support mult with Copy mode") - multiply-during-DMA is not available.

Sharding: data-parallel over batch B across 8 cores (4 batches/core),
constants replicated; cores fully independent (recurrence is only over T).
Per-core layout: [128, 512] per timestep, partition p = b_local*32 + sub,
free = n_low, neuron n = sub*512 + n_low.
"""

import sys

if "/opt/trn_rl_repo" not in sys.path:
    sys.path.insert(0, "/opt/trn_rl_repo")

import contextlib

import numpy as np

import concourse.bass as bass
import concourse.mybir as mybir
from concourse.bass_utils import run_bass_kernel_spmd

B, T, N = 32, 64, 16384
NCORES = 8
BL = B // NCORES
SUB = 32
NL = N // SUB  # 512
P = BL * SUB  # 128
TBLK = 8
NBLK = T // TBLK
BW = TBLK * NL  # 4096
F32 = mybir.dt.float32
F16 = mybir.dt.float16
F8 = mybir.dt.float8e4
ALU = mybir.AluOpType
AF = mybir.ActivationFunctionType

_CACHE: dict = {}


def _build_nc() -> bass.Bass:
    nc = bass.Bass()
    x = nc.dram_tensor("x", [BL, T, N], F32, kind="ExternalInput")
    cm_d = nc.dram_tensor("cm", [N], F32, kind="ExternalInput")
    sigth_d = nc.dram_tensor("sigth", [N], F32, kind="ExternalInput")
    thcm_d = nc.dram_tensor("thcm16", [N], F16, kind="ExternalInput")
    spikes8 = nc.dram_tensor("spikes8", [BL, T, N], F8, kind="ExternalOutput")
    mems16 = nc.dram_tensor("mems16", [BL, T, N], F16, kind="ExternalOutput")

    cm_2d = cm_d.rearrange("(s n) -> s n", n=NL)
    sigth_2d = sigth_d.rearrange("(s n) -> s n", n=NL)
    thcm_2d = thcm_d.rearrange("(s n) -> s n", n=NL)

    def x_src(b, k):
        return x[b, k * TBLK : (k + 1) * TBLK, :].rearrange(
            "t (s n) -> s t n", n=NL
        )

    def out_dst(dram, b, k):
        return dram[b, k * TBLK : (k + 1) * TBLK, :].rearrange(
            "t (s n) -> s t n", n=NL
        )

    def bv(tile, b):
        return tile[b * SUB : (b + 1) * SUB, :].rearrange(
            "p (t n) -> p t n", n=NL
        )

    with contextlib.ExitStack() as st:
        xb_all = st.enter_context(nc.sbuf_tensor([P, 3 * BW], F32))
        xs_t = st.enter_context(nc.sbuf_tensor([P, BW], F32))
        sigthb = st.enter_context(nc.sbuf_tensor([P, BW], F32))
        thcmb16 = st.enter_context(nc.sbuf_tensor([P, BW], F16))
        cm_t = st.enter_context(nc.sbuf_tensor([P, NL], F32))
        sigth_t = st.enter_context(nc.sbuf_tensor([P, NL], F32))
        thcm16_t = st.enter_context(nc.sbuf_tensor([P, NL], F16))
        uh_t = st.enter_context(nc.sbuf_tensor([P, 2 * NL], F32))
        w_all = st.enter_context(nc.sbuf_tensor([P, 2 * BW], F32))
        p_all = st.enter_context(nc.sbuf_tensor([P, 2 * BW], F32))
        p16_t = st.enter_context(nc.sbuf_tensor([P, BW], F16))
        m16_t = st.enter_context(nc.sbuf_tensor([P, BW], F16))
        s8_t = st.enter_context(nc.sbuf_tensor([P, BW], F8))
        c_sem = st.enter_context(nc.semaphore("c_sem"))
        rep_sem = st.enter_context(nc.semaphore("rep_sem"))
        xs0_sem = st.enter_context(nc.semaphore("xs0_sem"))
        xs1_sem = st.enter_context(nc.semaphore("xs1_sem"))
        xs2_sem = st.enter_context(nc.semaphore("xs2_sem"))
        xsd_sem = st.enter_context(nc.semaphore("xsd_sem"))
        w_sem = st.enter_context(nc.semaphore("w_sem"))
        pb_sem = st.enter_context(nc.semaphore("pb_sem"))
        p16_sem = st.enter_context(nc.semaphore("p16_sem"))
        spk_sem = st.enter_context(nc.semaphore("spk_sem"))
        m16d_sem = st.enter_context(nc.semaphore("m16d_sem"))
        mo_sem = st.enter_context(nc.semaphore("mo_sem"))
        so_sem = st.enter_context(nc.semaphore("so_sem"))
        block = st.enter_context(nc.Block())

        xslot_sems = [xs0_sem, xs1_sem, xs2_sem]

        def xb_r(k):
            return xb_all[:, (k % 3) * BW : (k % 3 + 1) * BW]

        def wsl(k, tl):
            r = k % 2
            return w_all[:, (r * TBLK + tl) * NL : (r * TBLK + tl + 1) * NL]

        def wblk(k):
            r = k % 2
            return w_all[:, r * BW : (r + 1) * BW]

        def psl(k, tl):
            r = k % 2
            return p_all[:, (r * TBLK + tl) * NL : (r * TBLK + tl + 1) * NL]

        def pblk(k):
            r = k % 2
            return p_all[:, r * BW : (r + 1) * BW]

        def uhsl(t):
            r = t % 2
            return uh_t[:, r * NL : (r + 1) * NL]

        @block.sync
        def _(sync):
            # const loads: replicate [NL]-chunked views to each 32-part group
            for src, dst in (
                (sigth_2d, sigth_t),
                (cm_2d, cm_t),
                (thcm_2d, thcm16_t),
            ):
                for b in range(BL):
                    sync.dma_start(
                        out=dst[b * SUB : (b + 1) * SUB, :], in_=src
                    ).then_inc(c_sem, 16)
            # input ring: first 3 blocks
            for k in range(min(3, NBLK)):
                for b in range(BL):
                    sync.dma_start(out=bv(xb_r(k), b), in_=x_src(b, k)).then_inc(
                        xslot_sems[k % 3], 16
                    )
            for k in range(NBLK):
                kf = k + 3
                if kf < NBLK:
                    # slot (kf%3) is free once xs-op of block kf-3 ran
                    sync.wait_ge(xsd_sem, kf - 3 + 1)
                    for b in range(BL):
                        sync.dma_start(
                            out=bv(xb_r(kf), b), in_=x_src(b, kf)
                        ).then_inc(xslot_sems[kf % 3], 16)
                # outputs of block k (spikes) and k-1 (mems)
                sync.wait_ge(spk_sem, k + 1)
                for b in range(BL):
                    sync.dma_start(
                        out=out_dst(spikes8, b, k), in_=bv(s8_t, b)
                    ).then_inc(so_sem, 16)
                if k >= 1:
                    sync.wait_ge(m16d_sem, k)
                    for b in range(BL):
                        sync.dma_start(
                            out=out_dst(mems16, b, k - 1), in_=bv(m16_t, b)
                        ).then_inc(mo_sem, 16)
            sync.wait_ge(m16d_sem, NBLK)
            for b in range(BL):
                sync.dma_start(
                    out=out_dst(mems16, b, NBLK - 1), in_=bv(m16_t, b)
                ).then_inc(mo_sem, 16)
            sync.wait_ge(so_sem, 64 * NBLK)
            sync.wait_ge(mo_sem, 64 * NBLK)

        @block.vector
        def _(vector):
            vector.wait_ge(c_sem, 16 * BL * 2)  # cm_t loaded
            vector.wait_ge(rep_sem, TBLK)  # sigthb tiled
            for k in range(NBLK):
                vector.wait_ge(xslot_sems[k % 3], 64 * (k // 3 + 1))
                if k >= 2:
                    # p ring slot k%2: ACT cast of block k-2 must be done
                    vector.wait_ge(p16_sem, k - 1)
                # xs for block k
                nc.vector.tensor_tensor(
                    out=xs_t[:, :], in0=xb_r(k), in1=sigthb[:, :], op=ALU.mult
                ).then_inc(xsd_sem, 1)
                if k >= 1:
                    # deferred add: uh_{8k} = p_{8k-1} + xs_{8k}
                    nc.vector.tensor_tensor(
                        out=uhsl(8 * k),
                        in0=psl(k - 1, TBLK - 1),
                        in1=xs_t[:, 0:NL],
                        op=ALU.add,
                    )
                    # mems16 for block k-1 (p16 cast by ACT during chain k-1)
                    vector.wait_ge(p16_sem, k)
                    if k >= 2:
                        vector.wait_ge(mo_sem, 64 * (k - 1))  # m16 tile WAR
                    nc.vector.tensor_tensor(
                        out=m16_t[:, :],
                        in0=p16_t[:, :],
                        in1=thcmb16[:, :],
                        op=ALU.mult,
                    ).then_inc(m16d_sem, 1)
                if k >= 2:
                    vector.wait_ge(spk_sem, k - 1)  # w ring WAR vs ACT exp
                for tl in range(TBLK):
                    t = k * TBLK + tl
                    uh = xs_t[:, 0:NL] if t == 0 else uhsl(t)
                    ins_w = nc.vector.scalar_tensor_tensor(
                        out=wsl(k, tl),
                        in0=uh,
                        scalar=1.0,
                        in1=cm_t[:, :],
                        op0=ALU.is_lt,
                        op1=ALU.mult,
                    )
                    if tl == TBLK - 1:
                        ins_w.then_inc(w_sem, 1)
                    ins_p = nc.vector.tensor_tensor(
                        out=psl(k, tl), in0=uh, in1=wsl(k, tl), op=ALU.mult
                    )
                    if tl == TBLK - 1:
                        ins_p.then_inc(pb_sem, 1)
                    if tl < TBLK - 1:
                        nc.vector.tensor_tensor(
                            out=uhsl(t + 1),
                            in0=psl(k, tl),
                            in1=xs_t[:, (tl + 1) * NL : (tl + 2) * NL],
                            op=ALU.add,
                        )
            # tail: mems16 for the last block
            vector.wait_ge(p16_sem, NBLK)
            vector.wait_ge(mo_sem, 64 * (NBLK - 1))
            nc.vector.tensor_tensor(
                out=m16_t[:, :],
                in0=p16_t[:, :],
                in1=thcmb16[:, :],
                op=ALU.mult,
            ).then_inc(m16d_sem, 1)

        @block.scalar
        def _(scalar):
            scalar.wait_ge(c_sem, 16 * BL)  # sigth_t loaded
            for tl in range(TBLK):
                nc.scalar.copy(
                    out=sigthb[:, tl * NL : (tl + 1) * NL], in_=sigth_t[:, :]
                ).then_inc(rep_sem, 1)
            scalar.wait_ge(c_sem, 16 * BL * 3)
            for tl in range(TBLK):
                nc.scalar.copy(
                    out=thcmb16[:, tl * NL : (tl + 1) * NL], in_=thcm16_t[:, :]
                ).then_inc(rep_sem, 1)
            for k in range(NBLK):
                # p block k -> fp16 (for the all-16-bit mems mult)
                scalar.wait_ge(pb_sem, k + 1)
                if k >= 1:
                    scalar.wait_ge(m16d_sem, k)  # p16 consumed by m16-op k-1
                nc.scalar.copy(out=p16_t[:, :], in_=pblk(k)).then_inc(
                    p16_sem, 1
                )
                # spikes block k: w==0 iff spike; exp(-1e30*w) = 1/0 exactly
                scalar.wait_ge(w_sem, k + 1)
                if k >= 1:
                    scalar.wait_ge(so_sem, 64 * k)  # s8 WAR
                nc.scalar.activation(
                    s8_t[:, :], wblk(k), AF.Exp, scale=-1e30
                ).then_inc(spk_sem, 1)

    return nc


def _get_nc() -> bass.Bass:
    if "nc" not in _CACHE:
        _CACHE["nc"] = _build_nc()
    return _CACHE["nc"]


def kernel(x, thresh, tau_x, _trace: bool = False, _tmpdir: str | None = None):
    x = np.ascontiguousarray(np.asarray(x, dtype=np.float32))
    thresh = np.ascontiguousarray(np.asarray(thresh, dtype=np.float32))
    tau_x = np.ascontiguousarray(np.asarray(tau_x, dtype=np.float32))
    assert x.shape == (B, T, N)

    # O(N) host-side constants; all O(B*T*N) math happens on-device.
    sig = (1.0 / (1.0 + np.exp(-tau_x.astype(np.float64)))).astype(np.float32)
    cm = (np.float32(1.0) - sig).astype(np.float32)
    sigth = (sig / thresh).astype(np.float32)
    thcm16 = (thresh / cm).astype(np.float16)

    nc = _get_nc()
    in_maps = [
        {
            "x": x[i * BL : (i + 1) * BL],
            "cm": cm,
            "sigth": sigth,
            "thcm16": thcm16,
        }
        for i in range(NCORES)
    ]
    res = run_bass_kernel_spmd(
        nc, in_maps, core_ids=list(range(NCORES)), trace=_trace, tmpdir=_tmpdir
    )
    spikes = np.concatenate(
        [np.asarray(r["spikes8"]).astype(np.float32) for r in res.results],
        axis=0,
    )
    mems = np.concatenate(
        [np.asarray(r["mems16"]).astype(np.float32) for r in res.results],
        axis=0,
    )
    if _trace:
        _CACHE["last_results"] = res
    return spikes, mems


# revision 12
# speedup vs baseline: 1.2231x; 1.2231x over previous
"""LIF neuron scan kernel v4 for Trainium2 (8 NeuronCores).

Reference recurrence per timestep t (elementwise over B x N):
    u = (x_t - m)*sig + m ; s = (u >= th) ; m = (1-s)*u
with sig = sigmoid(tau_x) in (0,1), th > 0 per neuron.

uh-space (v3) reformulation kept: uh = u/th, xs = x*(sig/th), cm = 1-sig:
    w  = (uh < 1) * cm        # stt, fp32 (w==0 iff spike; w carries the decay)
    p  = uh * w               # tt fp32
    uh'= p + xs'              # tt fp32
State stays fp32 end-to-end: fp16-state variants flip ~2-3k near-threshold
spikes (emulated: 2.2-2.6e-2 rel err > the 2e-2 gate), so 16-bit state is
not usable; only non-state paths are 16-bit.

v4 changes vs v3 (probe-informed; see probe.py):
  - Input pipeline: one 2 MB dma_start per 8-t block, 3-slot ring with a
    per-slot semaphore (no cross-block issue throttle). v3 serialized input
    DMAs behind output waits + a completion throttle and DVE idled 4.8 us
    per block waiting for x.
  - mems path: ACT casts the p block to fp16 (ACT is ~70% idle), DVE does
    mems16 = p16 * thcm16 as an all-fp16 tensor_tensor -> 2x_1P mode
    (2.2 us vs 4.3 us per block). mems output fp16 (was bf16): same bytes,
    ~8x better mantissa. Accuracy impact ~5e-4 rel, well inside the gate.
  - spikes: one ACT Exp per block ([128,4096], fp8e4 out, exact 0/1),
    halving spike DMA bytes and cutting per-t semaphore traffic.
  - Chain ops carry no per-t then_inc except the block-final ones: pure TT
    pitch is 602 ns; v3's per-op sem traffic ran it at 722 ns.
  - GpSimd compute offload was probed and rejected: a gpsimd tensor_tensor
    blocks concurrent DVE almost completely (one DVE op per gpsimd op).
    SWDGE DMA traffic does NOT block DVE (probed) but is not needed.
  - DMA accum_op=mult (CCE) is rejected by the compiler ("DMACopy does not
    support mult with Copy mode") - multiply-during-DMA is not available.

Sharding: data-parallel over batch B across 8 cores (4 batches/core),
constants replicated; cores fully independent (recurrence is only over T).
Per-core layout: [128, 512] per timestep, partition p = b_local*32 + sub,
free = n_low, neuron n = sub*512 + n_low.
"""

import sys

if "/opt/trn_rl_repo" not in sys.path:
    sys.path.insert(0, "/opt/trn_rl_repo")

import contextlib

import numpy as np

import concourse.bass as bass
import concourse.mybir as mybir
from concourse.bass_utils import run_bass_kernel_spmd

B, T, N = 32, 64, 16384
NCORES = 8
BL = B // NCORES
SUB = 32
NL = N // SUB  # 512
P = BL * SUB  # 128
TBLK = 8
NBLK = T // TBLK
BW = TBLK * NL  # 4096
F32 = mybir.dt.float32
F16 = mybir.dt.float16
F8 = mybir.dt.float8e4
ALU = mybir.AluOpType
AF = mybir.ActivationFunctionType

_CACHE: dict = {}


def _build_nc() -> bass.Bass:
    nc = bass.Bass()
    x = nc.dram_tensor("x", [BL, T, N], F32, kind="ExternalInput")
    cm_d = nc.dram_tensor("cm", [N], F32, kind="ExternalInput")
    sigth_d = nc.dram_tensor("sigth", [N], F32, kind="ExternalInput")
    thcm_d = nc.dram_tensor("thcm16", [N], F16, kind="ExternalInput")
    spikes8 = nc.dram_tensor("spikes8", [BL, T, N], F8, kind="ExternalOutput")
    mems16 = nc.dram_tensor("mems16", [BL, T, N], F16, kind="ExternalOutput")

    cm_2d = cm_d.rearrange("(s n) -> s n", n=NL)
    sigth_2d = sigth_d.rearrange("(s n) -> s n", n=NL)
    thcm_2d = thcm_d.rearrange("(s n) -> s n", n=NL)

    def x_src(b, k):
        return x[b, k * TBLK : (k + 1) * TBLK, :].rearrange(
            "t (s n) -> s t n", n=NL
        )

    def out_dst(dram, b, k):
        return dram[b, k * TBLK : (k + 1) * TBLK, :].rearrange(
            "t (s n) -> s t n", n=NL
        )

    def bv(tile, b):
        return tile[b * SUB : (b + 1) * SUB, :].rearrange(
            "p (t n) -> p t n", n=NL
        )

    with contextlib.ExitStack() as st:
        xb_all = st.enter_context(nc.sbuf_tensor([P, 3 * BW], F32))
        xs_t = st.enter_context(nc.sbuf_tensor([P, BW], F32))
        sigthb = st.enter_context(nc.sbuf_tensor([P, BW], F32))
        thcmb16 = st.enter_context(nc.sbuf_tensor([P, BW], F16))
        cm_t = st.enter_context(nc.sbuf_tensor([P, NL], F32))
        sigth_t = st.enter_context(nc.sbuf_tensor([P, NL], F32))
        thcm16_t = st.enter_context(nc.sbuf_tensor([P, NL], F16))
        uh_t = st.enter_context(nc.sbuf_tensor([P, 2 * NL], F32))
        w_all = st.enter_context(nc.sbuf_tensor([P, 2 * BW], F32))
        p_all = st.enter_context(nc.sbuf_tensor([P, 2 * BW], F32))
        p16_t = st.enter_context(nc.sbuf_tensor([P, BW], F16))
        m16_t = st.enter_context(nc.sbuf_tensor([P, BW], F16))
        s8_t = st.enter_context(nc.sbuf_tensor([P, BW], F8))
        c_sem = st.enter_context(nc.semaphore("c_sem"))
        rep_sem = st.enter_context(nc.semaphore("rep_sem"))
        xs0_sem = st.enter_context(nc.semaphore("xs0_sem"))
        xs1_sem = st.enter_context(nc.semaphore("xs1_sem"))
        xs2_sem = st.enter_context(nc.semaphore("xs2_sem"))
        xsd_sem = st.enter_context(nc.semaphore("xsd_sem"))
        w_sem = st.enter_context(nc.semaphore("w_sem"))
        pb_sem = st.enter_context(nc.semaphore("pb_sem"))
        p16_sem = st.enter_context(nc.semaphore("p16_sem"))
        spk_sem = st.enter_context(nc.semaphore("spk_sem"))
        m16d_sem = st.enter_context(nc.semaphore("m16d_sem"))
        mo_sem = st.enter_context(nc.semaphore("mo_sem"))
        so_sem = st.enter_context(nc.semaphore("so_sem"))
        block = st.enter_context(nc.Block())

        xslot_sems = [xs0_sem, xs1_sem, xs2_sem]

        def xb_r(k):
            return xb_all[:, (k % 3) * BW : (k % 3 + 1) * BW]

        def wsl(k, tl):
            r = k % 2
            return w_all[:, (r * TBLK + tl) * NL : (r * TBLK + tl + 1) * NL]

        def wblk(k):
            r = k % 2
            return w_all[:, r * BW : (r + 1) * BW]

        def psl(k, tl):
            r = k % 2
            return p_all[:, (r * TBLK + tl) * NL : (r * TBLK + tl + 1) * NL]

        def pblk(k):
            r = k % 2
            return p_all[:, r * BW : (r + 1) * BW]

        def uhsl(t):
            r = t % 2
            return uh_t[:, r * NL : (r + 1) * NL]

        @block.sync
        def _(sync):
            # x block 0 first (longest pole), then consts, then x1/x2
            for b in range(BL):
                sync.dma_start(out=bv(xb_r(0), b), in_=x_src(b, 0)).then_inc(
                    xslot_sems[0], 16
                )
            for src, dst in (
                (sigth_2d, sigth_t),
                (cm_2d, cm_t),
                (thcm_2d, thcm16_t),
            ):
                for b in range(BL):
                    sync.dma_start(
                        out=dst[b * SUB : (b + 1) * SUB, :], in_=src
                    ).then_inc(c_sem, 16)
            for k in (1, 2):
                for b in range(BL):
                    sync.dma_start(out=bv(xb_r(k), b), in_=x_src(b, k)).then_inc(
                        xslot_sems[k % 3], 16
                    )
            # mid-loop x input DMAs are issued from the ACT queue (the other
            # HWDGE ring) so they never serialize behind the output waits here
            for k in range(NBLK):
                # outputs of block k (spikes) and k-1 (mems)
                sync.wait_ge(spk_sem, k + 1)
                for b in range(BL):
                    sync.dma_start(
                        out=out_dst(spikes8, b, k), in_=bv(s8_t, b)
                    ).then_inc(so_sem, 16)
                if k >= 1:
                    sync.wait_ge(m16d_sem, k)
                    for b in range(BL):
                        sync.dma_start(
                            out=out_dst(mems16, b, k - 1), in_=bv(m16_t, b)
                        ).then_inc(mo_sem, 16)
            sync.wait_ge(m16d_sem, NBLK)
            for b in range(BL):
                sync.dma_start(
                    out=out_dst(mems16, b, NBLK - 1), in_=bv(m16_t, b)
                ).then_inc(mo_sem, 16)
            sync.wait_ge(so_sem, 64 * NBLK)
            sync.wait_ge(mo_sem, 64 * NBLK)

        @block.vector
        def _(vector):
            vector.wait_ge(c_sem, 16 * BL * 2)  # cm_t loaded
            vector.wait_ge(rep_sem, TBLK)  # sigthb tiled
            for k in range(NBLK):
                vector.wait_ge(xslot_sems[k % 3], 64 * (k // 3 + 1))
                if k >= 2:
                    # p ring slot k%2: ACT cast of block k-2 must be done
                    vector.wait_ge(p16_sem, k - 1)
                # xs for block k
                nc.vector.tensor_tensor(
                    out=xs_t[:, :], in0=xb_r(k), in1=sigthb[:, :], op=ALU.mult
                ).then_inc(xsd_sem, 1)
                if k >= 1:
                    # deferred add: uh_{8k} = p_{8k-1} + xs_{8k}
                    nc.vector.tensor_tensor(
                        out=uhsl(8 * k),
                        in0=psl(k - 1, TBLK - 1),
                        in1=xs_t[:, 0:NL],
                        op=ALU.add,
                    )
                if k >= 2:
                    vector.wait_ge(spk_sem, k - 1)  # w ring WAR vs ACT exp
                for tl in range(TBLK):
                    t = k * TBLK + tl
                    if k >= 1 and tl == 4:
                        # mems16 for block k-1, placed mid-chain so the ACT
                        # cast (done ~7us after chain k-1) is never waited on
                        vector.wait_ge(p16_sem, k)
                        if k >= 2:
                            vector.wait_ge(mo_sem, 64 * (k - 1))  # m16 WAR
                        nc.vector.tensor_tensor(
                            out=m16_t[:, :],
                            in0=p16_t[:, :],
                            in1=thcmb16[:, :],
                            op=ALU.mult,
                        ).then_inc(m16d_sem, 1)
                    uh = xs_t[:, 0:NL] if t == 0 else uhsl(t)
                    ins_w = nc.vector.scalar_tensor_tensor(
                        out=wsl(k, tl),
                        in0=uh,
                        scalar=1.0,
                        in1=cm_t[:, :],
                        op0=ALU.is_lt,
                        op1=ALU.mult,
                    )
                    if tl == TBLK - 1:
                        ins_w.then_inc(w_sem, 1)
                    ins_p = nc.vector.tensor_tensor(
                        out=psl(k, tl), in0=uh, in1=wsl(k, tl), op=ALU.mult
                    )
                    if tl == TBLK - 1:
                        ins_p.then_inc(pb_sem, 1)
                    if tl < TBLK - 1:
                        nc.vector.tensor_tensor(
                            out=uhsl(t + 1),
                            in0=psl(k, tl),
                            in1=xs_t[:, (tl + 1) * NL : (tl + 2) * NL],
                            op=ALU.add,
                        )
            # tail: mems16 for the last block
            vector.wait_ge(p16_sem, NBLK)
            vector.wait_ge(mo_sem, 64 * (NBLK - 1))
            nc.vector.tensor_tensor(
                out=m16_t[:, :],
                in0=p16_t[:, :],
                in1=thcmb16[:, :],
                op=ALU.mult,
            ).then_inc(m16d_sem, 1)

        @block.scalar
        def _(scalar):
            scalar.wait_ge(c_sem, 16 * BL)  # sigth_t loaded
            for tl in range(TBLK):
                nc.scalar.copy(
                    out=sigthb[:, tl * NL : (tl + 1) * NL], in_=sigth_t[:, :]
                ).then_inc(rep_sem, 1)
            scalar.wait_ge(c_sem, 16 * BL * 3)
            for tl in range(TBLK):
                nc.scalar.copy(
                    out=thcmb16[:, tl * NL : (tl + 1) * NL], in_=thcm16_t[:, :]
                ).then_inc(rep_sem, 1)
            for k in range(NBLK):
                # issue x input DMA for block k+3 (other HWDGE ring; gated
                # only on the xs-op that frees the ring slot)
                kf = k + 3
                if kf < NBLK:
                    scalar.wait_ge(xsd_sem, k + 1)
                    for b in range(BL):
                        nc.scalar.dma_start(
                            out=bv(xb_r(kf), b), in_=x_src(b, kf)
                        ).then_inc(xslot_sems[kf % 3], 16)
                # spikes block k: w==0 iff spike; exp(-1e30*w) = 1/0 exactly
                scalar.wait_ge(w_sem, k + 1)
                if k >= 1:
                    scalar.wait_ge(so_sem, 64 * k)  # s8 WAR
                nc.scalar.activation(
                    s8_t[:, :], wblk(k), AF.Exp, scale=-1e30
                ).then_inc(spk_sem, 1)
                # p block k -> fp16 (for the all-16-bit mems mult)
                scalar.wait_ge(pb_sem, k + 1)
                if k >= 1:
                    scalar.wait_ge(m16d_sem, k)  # p16 consumed by m16-op k-1
                nc.scalar.copy(out=p16_t[:, :], in_=pblk(k)).then_inc(
                    p16_sem, 1
                )

    return nc


def _get_nc() -> bass.Bass:
    if "nc" not in _CACHE:
        _CACHE["nc"] = _build_nc()
    return _CACHE["nc"]


def kernel(x, thresh, tau_x, _trace: bool = False, _tmpdir: str | None = None):
    x = np.ascontiguousarray(np.asarray(x, dtype=np.float32))
    thresh = np.ascontiguousarray(np.asarray(thresh, dtype=np.float32))
    tau_x = np.ascontiguousarray(np.asarray(tau_x, dtype=np.float32))
    assert x.shape == (B, T, N)

    # O(N) host-side constants; all O(B*T*N) math happens on-device.
    sig = (1.0 / (1.0 + np.exp(-tau_x.astype(np.float64)))).astype(np.float32)
    cm = (np.float32(1.0) - sig).astype(np.float32)
    sigth = (sig / thresh).astype(np.float32)
    thcm16 = (thresh / cm).astype(np.float16)

    nc = _get_nc()
    in_maps = [
        {
            "x": x[i * BL : (i + 1) * BL],
            "cm": cm,
            "sigth": sigth,
            "thcm16": thcm16,
        }
        for i in range(NCORES)
    ]
    res = run_bass_kernel_spmd(
        nc, in_maps, core_ids=list(range(NCORES)), trace=_trace, tmpdir=_tmpdir
    )
    spikes = np.concatenate(
        [np.asarray(r["spikes8"]).astype(np.float32) for r in res.results],
        axis=0,
    )
    mems = np.concatenate(
        [np.asarray(r["mems16"]).astype(np.float32) for r in res.results],
        axis=0,
    )
    if _trace:
        _CACHE["last_results"] = res
    return spikes, mems


# revision 19
# speedup vs baseline: 1.3999x; 1.1445x over previous
"""LIF neuron scan kernel v4 for Trainium2 (8 NeuronCores).

Reference recurrence per timestep t (elementwise over B x N):
    u = (x_t - m)*sig + m ; s = (u >= th) ; m = (1-s)*u
with sig = sigmoid(tau_x) in (0,1), th > 0 per neuron.

uh-space (v3) reformulation kept: uh = u/th, xs = x*(sig/th), cm = 1-sig:
    w  = (uh < 1) * cm        # stt, fp32 (w==0 iff spike; w carries the decay)
    p  = uh * w               # tt fp32
    uh'= p + xs'              # tt fp32
State stays fp32 end-to-end: fp16-state variants flip ~2-3k near-threshold
spikes (emulated: 2.2-2.6e-2 rel err > the 2e-2 gate), so 16-bit state is
not usable; only non-state paths are 16-bit.

v4 changes vs v3 (probe-informed; see probe.py):
  - Input pipeline: one 2 MB dma_start per 8-t block, 3-slot ring with a
    per-slot semaphore (no cross-block issue throttle). v3 serialized input
    DMAs behind output waits + a completion throttle and DVE idled 4.8 us
    per block waiting for x.
  - mems path: ACT casts the p block to fp16 (ACT is ~70% idle), DVE does
    mems16 = p16 * thcm16 as an all-fp16 tensor_tensor -> 2x_1P mode
    (2.2 us vs 4.3 us per block). mems output fp16 (was bf16): same bytes,
    ~8x better mantissa. Accuracy impact ~5e-4 rel, well inside the gate.
  - spikes: one ACT Exp per block ([128,4096], fp8e4 out, exact 0/1),
    halving spike DMA bytes and cutting per-t semaphore traffic.
  - Chain ops carry no per-t then_inc except the block-final ones: pure TT
    pitch is 602 ns; v3's per-op sem traffic ran it at 722 ns.
  - GpSimd compute offload was probed and rejected: a gpsimd tensor_tensor
    blocks concurrent DVE almost completely (one DVE op per gpsimd op).
    SWDGE DMA traffic does NOT block DVE (probed) but is not needed.
  - DMA accum_op=mult (CCE) is rejected by the compiler ("DMACopy does not
    support mult with Copy mode") - multiply-during-DMA is not available.

Sharding: data-parallel over batch B across 8 cores (4 batches/core),
constants replicated; cores fully independent (recurrence is only over T).
Per-core layout: [128, 512] per timestep, partition p = b_local*32 + sub,
free = n_low, neuron n = sub*512 + n_low.
"""

import sys

if "/opt/trn_rl_repo" not in sys.path:
    sys.path.insert(0, "/opt/trn_rl_repo")

import contextlib

import numpy as np

import concourse.bass as bass
import concourse.mybir as mybir
from concourse.bass_utils import run_bass_kernel_spmd

B, T, N = 32, 64, 16384
NCORES = 8
BL = B // NCORES
SUB = 32
NL = N // SUB  # 512
P = BL * SUB  # 128
TBLK = 8
NBLK = T // TBLK
BW = TBLK * NL  # 4096
F32 = mybir.dt.float32
F16 = mybir.dt.float16
F8 = mybir.dt.float8e4
ALU = mybir.AluOpType
AF = mybir.ActivationFunctionType

_CACHE: dict = {}


def _build_nc() -> bass.Bass:
    nc = bass.Bass()
    x = nc.dram_tensor("x", [BL, T, N], F32, kind="ExternalInput")
    cm_d = nc.dram_tensor("cm", [N], F32, kind="ExternalInput")
    sigth_d = nc.dram_tensor("sigth", [N], F32, kind="ExternalInput")
    thcm_d = nc.dram_tensor("thcm16", [N], F16, kind="ExternalInput")
    spikes8 = nc.dram_tensor("spikes8", [BL, T, N], F8, kind="ExternalOutput")
    mems16 = nc.dram_tensor("mems16", [BL, T, N], F16, kind="ExternalOutput")

    cm_2d = cm_d.rearrange("(s n) -> s n", n=NL)
    sigth_2d = sigth_d.rearrange("(s n) -> s n", n=NL)
    thcm_2d = thcm_d.rearrange("(s n) -> s n", n=NL)

    def x_src(b, k):
        return x[b, k * TBLK : (k + 1) * TBLK, :].rearrange(
            "t (s n) -> s t n", n=NL
        )

    def out_dst(dram, b, k):
        return dram[b, k * TBLK : (k + 1) * TBLK, :].rearrange(
            "t (s n) -> s t n", n=NL
        )

    def bv(tile, b):
        return tile[b * SUB : (b + 1) * SUB, :].rearrange(
            "p (t n) -> p t n", n=NL
        )

    with contextlib.ExitStack() as st:
        xb_all = st.enter_context(nc.sbuf_tensor([P, 3 * BW], F32))
        xs_t = st.enter_context(nc.sbuf_tensor([P, BW], F32))
        sigthb = st.enter_context(nc.sbuf_tensor([P, BW], F32))
        thcmb16 = st.enter_context(nc.sbuf_tensor([P, BW], F16))
        cm_t = st.enter_context(nc.sbuf_tensor([P, NL], F32))
        sigth_t = st.enter_context(nc.sbuf_tensor([P, NL], F32))
        thcm16_t = st.enter_context(nc.sbuf_tensor([P, NL], F16))
        uh_t = st.enter_context(nc.sbuf_tensor([P, 2 * NL], F32))
        w_all = st.enter_context(nc.sbuf_tensor([P, 2 * BW], F32))
        p_all = st.enter_context(nc.sbuf_tensor([P, 2 * BW], F32))
        p16_t = st.enter_context(nc.sbuf_tensor([P, BW], F16))
        m16_all = st.enter_context(nc.sbuf_tensor([P, 2 * BW], F16))
        s8_t = st.enter_context(nc.sbuf_tensor([P, BW], F8))
        c_sem = st.enter_context(nc.semaphore("c_sem"))
        rep_sem = st.enter_context(nc.semaphore("rep_sem"))
        xs0_sem = st.enter_context(nc.semaphore("xs0_sem"))
        xs1_sem = st.enter_context(nc.semaphore("xs1_sem"))
        xs2_sem = st.enter_context(nc.semaphore("xs2_sem"))
        xsd_sem = st.enter_context(nc.semaphore("xsd_sem"))
        w_sem = st.enter_context(nc.semaphore("w_sem"))
        pb_sem = st.enter_context(nc.semaphore("pb_sem"))
        p16_sem = st.enter_context(nc.semaphore("p16_sem"))
        spk_sem = st.enter_context(nc.semaphore("spk_sem"))
        m16d_sem = st.enter_context(nc.semaphore("m16d_sem"))
        mo_sem = st.enter_context(nc.semaphore("mo_sem"))
        so_sem = st.enter_context(nc.semaphore("so_sem"))
        block = st.enter_context(nc.Block())

        xslot_sems = [xs0_sem, xs1_sem, xs2_sem]

        def xb_r(k):
            return xb_all[:, (k % 3) * BW : (k % 3 + 1) * BW]

        def wsl(k, tl):
            r = k % 2
            return w_all[:, (r * TBLK + tl) * NL : (r * TBLK + tl + 1) * NL]

        def wblk(k):
            r = k % 2
            return w_all[:, r * BW : (r + 1) * BW]

        def psl(k, tl):
            r = k % 2
            return p_all[:, (r * TBLK + tl) * NL : (r * TBLK + tl + 1) * NL]

        def pblk(k):
            r = k % 2
            return p_all[:, r * BW : (r + 1) * BW]

        def uhsl(t):
            r = t % 2
            return uh_t[:, r * NL : (r + 1) * NL]

        def m16sl(j):
            r = j % 2
            return m16_all[:, r * BW : (r + 1) * BW]

        @block.sync
        def _(sync):
            # consts first (tiny, unblock ACT replication), then x0/x1/x2
            for src, dst in (
                (sigth_2d, sigth_t),
                (cm_2d, cm_t),
                (thcm_2d, thcm16_t),
            ):
                for b in range(BL):
                    sync.dma_start(
                        out=dst[b * SUB : (b + 1) * SUB, :], in_=src
                    ).then_inc(c_sem, 16)
            for k in (0, 1, 2):
                for b in range(BL):
                    sync.dma_start(out=bv(xb_r(k), b), in_=x_src(b, k)).then_inc(
                        xslot_sems[k % 3], 16
                    )
            # mid-loop x input DMAs are issued from the ACT queue (the other
            # HWDGE ring) so they never serialize behind the output waits here
            for k in range(NBLK):
                # outputs of block k (spikes) and k-1 (mems)
                sync.wait_ge(spk_sem, k + 1)
                for b in range(BL):
                    sync.dma_start(
                        out=out_dst(spikes8, b, k), in_=bv(s8_t, b)
                    ).then_inc(so_sem, 16)
                if k >= 1:
                    sync.wait_ge(m16d_sem, k)
                    for b in range(BL):
                        sync.dma_start(
                            out=out_dst(mems16, b, k - 1),
                            in_=bv(m16sl(k - 1), b),
                        ).then_inc(mo_sem, 16)
            sync.wait_ge(m16d_sem, NBLK)
            for b in range(BL):
                sync.dma_start(
                    out=out_dst(mems16, b, NBLK - 1),
                    in_=bv(m16sl(NBLK - 1), b),
                ).then_inc(mo_sem, 16)
            sync.wait_ge(so_sem, 64 * NBLK)
            sync.wait_ge(mo_sem, 64 * NBLK)

        @block.vector
        def _(vector):
            vector.wait_ge(c_sem, 16 * BL * 2)  # cm_t loaded
            vector.wait_ge(rep_sem, TBLK)  # sigthb tiled
            for k in range(NBLK):
                vector.wait_ge(xslot_sems[k % 3], 64 * (k // 3 + 1))
                if k >= 2:
                    # p ring slot k%2: ACT cast of block k-2 must be done
                    vector.wait_ge(p16_sem, k - 1)
                # xs for block k
                nc.vector.tensor_tensor(
                    out=xs_t[:, :], in0=xb_r(k), in1=sigthb[:, :], op=ALU.mult
                ).then_inc(xsd_sem, 1)
                if k >= 1:
                    # deferred add: uh_{8k} = p_{8k-1} + xs_{8k}
                    nc.vector.tensor_tensor(
                        out=uhsl(8 * k),
                        in0=psl(k - 1, TBLK - 1),
                        in1=xs_t[:, 0:NL],
                        op=ALU.add,
                    )
                if k >= 2:
                    vector.wait_ge(spk_sem, k - 1)  # w ring WAR vs ACT exp
                for tl in range(TBLK):
                    t = k * TBLK + tl
                    if k >= 1 and tl == 4:
                        # mems16 for block k-1, placed mid-chain so the ACT
                        # cast (done ~7us after chain k-1) is never waited on
                        vector.wait_ge(p16_sem, k)
                        if k >= 3:
                            vector.wait_ge(mo_sem, 64 * (k - 2))  # m16 WAR
                        nc.vector.tensor_tensor(
                            out=m16sl(k - 1),
                            in0=p16_t[:, :],
                            in1=thcmb16[:, :],
                            op=ALU.mult,
                        ).then_inc(m16d_sem, 1)
                    uh = xs_t[:, 0:NL] if t == 0 else uhsl(t)
                    ins_w = nc.vector.scalar_tensor_tensor(
                        out=wsl(k, tl),
                        in0=uh,
                        scalar=1.0,
                        in1=cm_t[:, :],
                        op0=ALU.is_lt,
                        op1=ALU.mult,
                    )
                    if tl == TBLK - 1:
                        ins_w.then_inc(w_sem, 1)
                    ins_p = nc.vector.tensor_tensor(
                        out=psl(k, tl), in0=uh, in1=wsl(k, tl), op=ALU.mult
                    )
                    if tl == TBLK - 1:
                        ins_p.then_inc(pb_sem, 1)
                    if tl < TBLK - 1:
                        nc.vector.tensor_tensor(
                            out=uhsl(t + 1),
                            in0=psl(k, tl),
                            in1=xs_t[:, (tl + 1) * NL : (tl + 2) * NL],
                            op=ALU.add,
                        )
            # tail: mems16 for the last block, straight from fp32 p (mixed
            # dtype TT, ~4.4us) -- skips waiting for the ACT cast
            vector.wait_ge(mo_sem, 64 * (NBLK - 2))
            nc.vector.tensor_tensor(
                out=m16sl(NBLK - 1),
                in0=pblk(NBLK - 1),
                in1=thcmb16[:, :],
                op=ALU.mult,
            ).then_inc(m16d_sem, 1)

        @block.scalar
        def _(scalar):
            scalar.wait_ge(c_sem, 16 * BL)  # sigth_t loaded
            for tl in range(TBLK):
                nc.scalar.copy(
                    out=sigthb[:, tl * NL : (tl + 1) * NL], in_=sigth_t[:, :]
                ).then_inc(rep_sem, 1)
            scalar.wait_ge(c_sem, 16 * BL * 3)
            for tl in range(TBLK):
                nc.scalar.copy(
                    out=thcmb16[:, tl * NL : (tl + 1) * NL], in_=thcm16_t[:, :]
                ).then_inc(rep_sem, 1)
            for k in range(NBLK):
                # spikes block k first (gates DVE's w ring + sync's s8-out):
                # w==0 iff spike; exp(-1e30*w) = 1/0 exactly
                scalar.wait_ge(w_sem, k + 1)
                if k >= 1:
                    scalar.wait_ge(so_sem, 64 * k)  # s8 WAR
                nc.scalar.activation(
                    s8_t[:, :], wblk(k), AF.Exp, scale=-1e30
                ).then_inc(spk_sem, 1)
                # issue x input DMA for block k+3 (other HWDGE ring; gated
                # only on the xs-op that frees the ring slot)
                kf = k + 3
                if kf < NBLK:
                    scalar.wait_ge(xsd_sem, k + 1)
                    for b in range(BL):
                        nc.scalar.dma_start(
                            out=bv(xb_r(kf), b), in_=x_src(b, kf)
                        ).then_inc(xslot_sems[kf % 3], 16)
                # p block k -> fp16 (for the all-16-bit mems mult); skipped
                # for the last block (its mems come straight from fp32 p)
                if k < NBLK - 1:
                    scalar.wait_ge(pb_sem, k + 1)
                    if k >= 1:
                        scalar.wait_ge(m16d_sem, k)  # p16 used by m16-op k-1
                    nc.scalar.copy(out=p16_t[:, :], in_=pblk(k)).then_inc(
                        p16_sem, 1
                    )

    return nc


def _get_nc() -> bass.Bass:
    if "nc" not in _CACHE:
        _CACHE["nc"] = _build_nc()
    return _CACHE["nc"]


def kernel(x, thresh, tau_x, _trace: bool = False, _tmpdir: str | None = None):
    x = np.ascontiguousarray(np.asarray(x, dtype=np.float32))
    thresh = np.ascontiguousarray(np.asarray(thresh, dtype=np.float32))
    tau_x = np.ascontiguousarray(np.asarray(tau_x, dtype=np.float32))
    assert x.shape == (B, T, N)

    # O(N) host-side constants; all O(B*T*N) math happens on-device.
    sig = (1.0 / (1.0 + np.exp(-tau_x.astype(np.float64)))).astype(np.float32)
    cm = (np.float32(1.0) - sig).astype(np.float32)
    sigth = (sig / thresh).astype(np.float32)
    thcm16 = (thresh / cm).astype(np.float16)

    nc = _get_nc()
    in_maps = [
        {
            "x": x[i * BL : (i + 1) * BL],
            "cm": cm,
            "sigth": sigth,
            "thcm16": thcm16,
        }
        for i in range(NCORES)
    ]
    res = run_bass_kernel_spmd(
        nc, in_maps, core_ids=list(range(NCORES)), trace=_trace, tmpdir=_tmpdir
    )
    spikes = np.concatenate(
        [np.asarray(r["spikes8"]).astype(np.float32) for r in res.results],
        axis=0,
    )
    mems = np.concatenate(
        [np.asarray(r["mems16"]).astype(np.float32) for r in res.results],
        axis=0,
    )
    if _trace:
        _CACHE["last_results"] = res
    return spikes, mems
